# revision 3
# baseline (speedup 1.0000x reference)
"""Trainium2 Bass kernel for causal GQA self-attention (B=2, T=2048, C=2048,
Hq=16, Hkv=4, d=128, RoPE base 1e6).

Sharding: 8 cores = 2 batches x 4 kv-head groups. Each core computes, for its
(batch b, kv group g): the q/k/v projections restricted to that group (4 q
heads + 1 kv head), RoPE, causal attention, and the partial o_proj
(y_group @ Wo[group rows]). The host sums the 4 partial o_proj outputs per
batch (the all-reduce/unshard step of tensor parallelism).

Device layout notes:
  - all activations/weights are fp16 (better mantissa than bf16, same PE
    speed); inputs are pre-chunked on the host to [128, ...] layouts whose
    DRAM rows are >=4KB contiguous, so every DMA is ~128 descriptors.
  - k/v projections run contraction-outer over 8 PSUM banks so PE starts as
    soon as the first x chunk lands instead of waiting for the full 8MB.
  - scores are computed transposed (S^T = k @ qT) per 128-k-block pair;
    softmax skips max-subtraction; exp row-sums are accumulated on DVE in
    fp16 and reduced with a single [1,512] ones-matmul per (head, q-chunk)
    (the baseline spent 44us of PE on per-block ones-matmuls).
  - diagonal 128-blocks narrow the scores/AV matmuls to the unmasked columns;
    the skipped columns of pt are never read so no memset is needed.
  - 1/rowsum via DVE reciprocal; broadcast across partitions via a DRAM
    round-trip; normalization applied to the evicted (unnormalized) y^T.
  - the v bias is folded out entirely: since softmax rows sum to 1, bv
    contributes the constant row bv_tiled @ Wo_g, added on the host.
  - o_proj for q-chunk qc is emitted between head 0 and head 1 of q-chunk
    qc-1 so the rowsum/normalize latency of every head overlaps PE work.
  - attention is software-pipelined: scores for block-pair g+1 are issued
    before AV of pair g, so PE never idles on the exp (ACT) latency and
    stays at its max p-state clock.
"""

import numpy as np

import concourse.bass as bass
import concourse.mybir as mybir
from concourse import bacc
from concourse.tile import TileContext
from concourse.bass_utils import run_bass_kernel_spmd
from concourse.masks import make_identity

F16 = mybir.dt.float16
F32 = mybir.dt.float32

T = 2048
C = 2048
D = 128
NH = 4           # q heads per core
CI = C // 128    # contraction chunks
TC = T // 512    # t chunks of 512
TB = T // 128    # t blocks of 128
SCALE = 1.0 / np.sqrt(D)

_PROGRAM = None


def _ts(i, s):
    return bass.ts(i, s)


def _patch_act_tables():
    """Force every ACT function this kernel uses to resolve to the
    natural_log_exp_and_others table set, so the whole kernel needs exactly
    one ACT_TABLE_LOAD. Returns an undo callable."""
    import concourse.bacc as bacc_mod

    orig = bacc_mod.get_activation_tables
    A = mybir.ActivationFunctionType
    mine = {A.Exp, A.Ln, A.Identity, A.Copy}

    def patched(arch):
        tables = dict(orig(arch))
        for name in tables:
            if name != "natural_log_exp_and_others":
                tables[name] = set(tables[name]) - mine
        return tables

    bacc_mod.get_activation_tables = patched

    def undo():
        bacc_mod.get_activation_tables = orig

    return undo


def _build_program():
    undo = _patch_act_tables()
    try:
        return _build_program_inner()
    finally:
        undo()


def _build_program_inner():
    nc = bacc.Bacc("TRN2", target_bir_lowering=False, debug=False, num_devices=8)

    # DRAM inputs, all pre-chunked on host so partition rows are contiguous
    xT_d = nc.dram_tensor("xT", [128, CI, T], F16, kind="ExternalInput").ap()
    wq_d = nc.dram_tensor("wq", [128, CI, NH * D], F16, kind="ExternalInput").ap()
    wk_d = nc.dram_tensor("wk", [128, CI, D], F16, kind="ExternalInput").ap()
    wv_d = nc.dram_tensor("wv", [128, CI, D], F16, kind="ExternalInput").ap()
    wo_d = nc.dram_tensor("wo", [128, NH, C], F16, kind="ExternalInput").ap()
    bq_d = nc.dram_tensor("bq", [D, NH], F32, kind="ExternalInput").ap()
    bk_d = nc.dram_tensor("bk", [D, 1], F32, kind="ExternalInput").ap()
    cos_d = nc.dram_tensor("cosT", [D, T], F16, kind="ExternalInput").ap()
    sin_d = nc.dram_tensor("sinT", [D, T], F16, kind="ExternalInput").ap()
    tri_d = nc.dram_tensor("tri", [D, D], F16, kind="ExternalInput").ap()
    out_d = nc.dram_tensor("out", [T, C], F16, kind="ExternalOutput").ap()
    # scratch for the per-(head, q-chunk) 1/rowsum broadcast round-trip
    sscr = nc.dram_tensor("sscr", [NH * TC, 512], F32).ap()

    Ident = mybir.ActivationFunctionType.Identity
    Exp = mybir.ActivationFunctionType.Exp

    with TileContext(nc) as tc:
        with (
            tc.tile_pool(name="consts", bufs=1) as consts,
            tc.tile_pool(name="acts", bufs=1) as acts,
        ):
            # ---- resident constants; DMA issue order == first-use order ---
            wk_sb = consts.tile([128, CI, D], F16)
            nc.sync.dma_start(out=wk_sb[:], in_=wk_d[:])
            wv_sb = consts.tile([128, CI, D], F16)
            nc.sync.dma_start(out=wv_sb[:], in_=wv_d[:])
            xT_sb = consts.tile([128, CI, T], F16)
            for ci in range(CI):
                nc.sync.dma_start(out=xT_sb[:, ci, :], in_=xT_d[:, ci, :])
            bk_sb = consts.tile([128, 1], F32)
            nc.sync.dma_start(out=bk_sb[:], in_=bk_d[:])
            bq_sb = consts.tile([128, NH], F32)
            nc.sync.dma_start(out=bq_sb[:], in_=bq_d[:])
            cos_sb = consts.tile([128, T], F16)
            nc.sync.dma_start(out=cos_sb[:], in_=cos_d[:])
            sin_sb = consts.tile([128, T], F16)
            nc.sync.dma_start(out=sin_sb[:], in_=sin_d[:])
            wq_sb = consts.tile([128, CI, NH * D], F16)
            nc.sync.dma_start(out=wq_sb[:], in_=wq_d[:])
            tri_sb = consts.tile([128, 128], F16)
            nc.sync.dma_start(out=tri_sb[:], in_=tri_d[:])
            wo_sb = consts.tile([128, NH, C], F16)
            nc.sync.dma_start(out=wo_sb[:], in_=wo_d[:])
            ones_sb = consts.tile([128, 1], F16)
            nc.vector.memset(ones_sb[:], 1.0)
            ident_sb = consts.tile([128, 128], F16)
            make_identity(nc, ident_sb[:])

            # ---- persistent activations ---------------------------------
            qT_all = acts.tile([128, NH, T], F16)    # rotated q^T per head
            kT_all = acts.tile([128, T], F16)        # rotated k^T
            v_sb = acts.tile([128, TB, D], F16)      # v in natural [t, d] blocks
            vbb = acts.tile([128, TC, 512], F16)     # v^T staging for transpose
            yTn = acts.tile([128, NH, T], F16)       # normalized y^T per head

            # ---- phase 1a: k+v projections, contraction-outer ------------
            # 8 live PSUM accumulations; matmul(ci) only needs x chunk ci,
            # so PE starts ~2us in and overlaps the rest of the x load.
            with (
                tc.tile_pool(name="pkv", bufs=1, space="PSUM") as pkv,
                tc.tile_pool(name="ev1", bufs=2) as ev1,
            ):
                pskv = [pkv.tile([128, 512], F32, name=f"pskv{i}") for i in range(8)]
                for ci in range(CI):
                    for t4 in range(TC):
                        nc.tensor.matmul(
                            pskv[t4][:],
                            wk_sb[:, ci, :],
                            xT_sb[:, ci, _ts(t4, 512)],
                            start=(ci == 0),
                            stop=(ci == CI - 1),
                        )
                    for t4 in range(TC):
                        nc.tensor.matmul(
                            pskv[4 + t4][:],
                            wv_sb[:, ci, :],
                            xT_sb[:, ci, _ts(t4, 512)],
                            start=(ci == 0),
                            stop=(ci == CI - 1),
                        )
                for t4 in range(TC):
                    # k: bias + rope
                    kb16 = ev1.tile([128, 512], F16, tag="kb")
                    nc.scalar.activation(kb16[:], pskv[t4][:], Ident, bias=bk_sb[:, 0:1])
                    sh = ev1.tile([128, 512], F16, tag="sh")
                    nc.sync.dma_start(out=sh[0:64, :], in_=kb16[64:128, :])
                    nc.sync.dma_start(out=sh[64:128, :], in_=kb16[0:64, :])
                    t1 = ev1.tile([128, 512], F16, tag="t1")
                    nc.vector.tensor_mul(t1[:], kb16[:], cos_sb[:, _ts(t4, 512)])
                    nc.vector.tensor_mul(sh[:], sh[:], sin_sb[:, _ts(t4, 512)])
                    nc.vector.tensor_add(kT_all[:, _ts(t4, 512)], t1[:], sh[:])
                    # v: no bias on device (bv folded into host constant row)
                    nc.vector.tensor_copy(vbb[:, t4, :], pskv[4 + t4][:])

            # ---- phase 1b: v transposes + q projections + rope -----------
            with (
                tc.tile_pool(name="pq", bufs=2, space="PSUM") as pq,
                tc.tile_pool(name="vt", bufs=2, space="PSUM") as vtp,
                tc.tile_pool(name="ev2", bufs=3) as ev2,
            ):
                for tb in range(TB):
                    t4, jj = divmod(tb, 4)
                    vt_ps = vtp.tile([128, 128], F16)
                    nc.tensor.transpose(vt_ps[:], vbb[:, t4, _ts(jj, 128)], ident_sb[:])
                    nc.vector.tensor_copy(v_sb[:, tb, :], vt_ps[:])
                for m in range(NH):
                    for t4 in range(TC):
                        ps = pq.tile([128, 512], F32)
                        for ci in range(CI):
                            nc.tensor.matmul(
                                ps[:],
                                wq_sb[:, ci, _ts(m, 128)],
                                xT_sb[:, ci, _ts(t4, 512)],
                                start=(ci == 0),
                                stop=(ci == CI - 1),
                            )
                        qb = ev2.tile([128, 512], F16, tag="qb")
                        nc.scalar.activation(qb[:], ps[:], Ident, bias=bq_sb[:, m : m + 1])
                        sh = ev2.tile([128, 512], F16, tag="sh")
                        nc.sync.dma_start(out=sh[0:64, :], in_=qb[64:128, :])
                        nc.sync.dma_start(out=sh[64:128, :], in_=qb[0:64, :])
                        t1 = ev2.tile([128, 512], F16, tag="t1")
                        nc.vector.tensor_mul(t1[:], qb[:], cos_sb[:, _ts(t4, 512)])
                        nc.vector.tensor_mul(sh[:], sh[:], sin_sb[:, _ts(t4, 512)])
                        nc.vector.tensor_add(qT_all[:, m, _ts(t4, 512)], t1[:], sh[:])

            # ---- phase 2: attention + interleaved o_proj -----------------
            with (
                tc.tile_pool(name="st", bufs=2, space="PSUM") as stp,
                tc.tile_pool(name="yt", bufs=1, space="PSUM") as ytp,
                tc.tile_pool(name="rs", bufs=1, space="PSUM") as rsp,
                tc.tile_pool(name="po", bufs=2, space="PSUM") as pop,
                tc.tile_pool(name="ptp", bufs=4) as ptp,
                tc.tile_pool(name="sacc", bufs=2) as saccp,
                tc.tile_pool(name="ytu", bufs=2) as ytup,
                tc.tile_pool(name="siv", bufs=2) as sivp,
                tc.tile_pool(name="sb", bufs=3) as sbp,
                tc.tile_pool(name="oe", bufs=2) as oep,
            ):

                def emit_oproj(qc):
                    for ti in range(4 * qc, 4 * qc + 4):
                        oe = oep.tile([128, C], F16)
                        for nj in range(TC):
                            po = pop.tile([128, 512], F32)
                            for hh in range(NH):
                                nc.tensor.matmul(
                                    po[:],
                                    yTn[:, hh, _ts(ti, 128)],
                                    wo_sb[:, hh, _ts(nj, 512)],
                                    start=(hh == 0),
                                    stop=(hh == NH - 1),
                                )
                            if nj % 2 == 0:
                                nc.vector.tensor_copy(oe[:, _ts(nj, 512)], po[:])
                            else:
                                nc.scalar.copy(oe[:, _ts(nj, 512)], po[:])
                        nc.sync.dma_start(out=out_d[_ts(ti, 128), :], in_=oe[:])

                def emit_scores_pair(g, h, qc):
                    """Scores for k-blocks 2g, 2g+1 against q-chunk qc of head
                    h, narrowed to unmasked columns on diagonal blocks."""
                    st = stp.tile([128, 1024], F32)
                    for u in range(2):
                        kb = 2 * g + u
                        j = kb - 4 * qc
                        off = max(j, 0) * 128
                        nc.tensor.matmul(
                            st[:, u * 512 + off : (u + 1) * 512],
                            kT_all[:, _ts(kb, 128)],
                            qT_all[:, h, qc * 512 + off : (qc + 1) * 512],
                            start=True,
                            stop=True,
                        )
                    return st

                pending_oproj = None
                for qc in (3, 2, 1, 0):
                    for h in range(NH):
                        nkb = 4 * (qc + 1)
                        ng = nkb // 2
                        yt_ps = ytp.tile([128, 512], F32)
                        sacc = saccp.tile([128, 512], F16)
                        sts = [emit_scores_pair(0, h, qc)]
                        if ng > 1:
                            sts.append(emit_scores_pair(1, h, qc))
                        for g in range(ng):
                            st = sts[g % 2]
                            pt = ptp.tile([128, 1024], F16)
                            nc.scalar.activation(pt[:], st[:], Exp, scale=SCALE)
                            if g + 2 < ng:
                                sts[g % 2] = emit_scores_pair(g + 2, h, qc)
                            for u in range(2):
                                kb = 2 * g + u
                                j = kb - 4 * qc
                                off = max(j, 0) * 128
                                if j >= 0:
                                    blk = pt[:, u * 512 + j * 128 : u * 512 + (j + 1) * 128]
                                    nc.vector.tensor_mul(blk, blk, tri_sb[:])
                                src = pt[:, u * 512 + off : (u + 1) * 512]
                                if kb == 0:
                                    nc.vector.tensor_copy(sacc[:], pt[:, 0:512])
                                else:
                                    nc.vector.tensor_add(
                                        sacc[:, off:512], sacc[:, off:512], src
                                    )
                                nc.tensor.matmul(
                                    yt_ps[:, off:512],
                                    v_sb[:, kb, :],
                                    src,
                                    start=(kb == 0),
                                    stop=(kb == nkb - 1),
                                    skip_group_check=True,
                                )
                        # rowsum via a single ones-matmul over the DVE accum
                        rs_ps = rsp.tile([1, 512], F32)
                        nc.tensor.matmul(
                            rs_ps[:], ones_sb[:], sacc[:], start=True, stop=True
                        )
                        # evict yT unnormalized right away (frees PSUM bank)
                        ytu = ytup.tile([128, 512], F16)
                        nc.vector.tensor_copy(ytu[:], yt_ps[:])
                        si = sivp.tile([1, 512], F32)
                        nc.vector.reciprocal(si[:], rs_ps[:])
                        idx = h * TC + qc
                        nc.sync.dma_start(out=sscr[idx : idx + 1, :], in_=si[:])
                        sb = sbp.tile([128, 512], F32)
                        row = sscr[idx : idx + 1, :]
                        bc = bass.AP(
                            tensor=row.tensor,
                            offset=row.offset,
                            ap=[[0, 128]] + row.ap[1:],
                        )
                        nc.sync.dma_start(out=sb[:], in_=bc)
                        nc.vector.tensor_mul(yTn[:, h, _ts(qc, 512)], ytu[:], sb[:])
                        # o_proj of the previous qc overlaps later heads
                        if h == 0 and pending_oproj is not None:
                            emit_oproj(pending_oproj)
                            pending_oproj = None
                    pending_oproj = qc
                emit_oproj(pending_oproj)

    nc.finalize()
    return nc


def _get_program():
    global _PROGRAM
    if _PROGRAM is None:
        _PROGRAM = _build_program()
    return _PROGRAM


def _rope_tables():
    inv_freq = 1.0 / (1000000.0 ** (np.arange(0, D, 2, dtype=np.float64) / D))
    pos = np.arange(T, dtype=np.float64)
    si = np.outer(pos, inv_freq)                      # [T, D/2]
    cos_h, sin_h = np.cos(si), np.sin(si)
    cos = np.stack([cos_h, cos_h], axis=-1).reshape(T, D)
    sin = np.stack([sin_h, sin_h], axis=-1).reshape(T, D)
    cosT = np.ascontiguousarray(cos.T)                # [D, T]
    sinT = np.ascontiguousarray(sin.T)
    # rotate-half as a partition shift: sh[i<64]=q[i+64], sh[i>=64]=q[i-64];
    # q_rot = q*cos + sh*sin_signed with the -1 for i<64 baked into the table
    sinT[: D // 2] *= -1.0
    return cosT, sinT


def make_in_maps(x, Wq, bq, Wk, bk, Wv, bv, Wo):
    f16 = np.float16
    cosT, sinT = _rope_tables()
    tri = np.triu(np.ones((D, D))).astype(f16)        # [k, q]: keep q >= k
    cosT, sinT = cosT.astype(f16), sinT.astype(f16)

    def chunk(w, width):   # [C, width] -> [128, CI, width]
        return np.ascontiguousarray(
            np.asarray(w, np.float32).reshape(CI, 128, width).transpose(1, 0, 2)
        ).astype(f16)

    in_maps = []
    for b in range(2):
        xTc = np.ascontiguousarray(
            np.asarray(x[b], np.float32).T.reshape(CI, 128, T).transpose(1, 0, 2)
        ).astype(f16)
        for g in range(4):
            wog = np.ascontiguousarray(
                np.asarray(Wo[g * 512 : (g + 1) * 512, :], np.float32)
                .reshape(NH, 128, C)
                .transpose(1, 0, 2)
            ).astype(f16)
            in_maps.append(
                {
                    "xT": xTc,
                    "wq": chunk(Wq[:, g * 512 : (g + 1) * 512], 512),
                    "wk": chunk(Wk[:, g * 128 : (g + 1) * 128], 128),
                    "wv": chunk(Wv[:, g * 128 : (g + 1) * 128], 128),
                    "wo": wog,
                    "bq": np.ascontiguousarray(
                        np.asarray(bq[g * 512 : (g + 1) * 512], np.float32)
                        .reshape(NH, D)
                        .T
                    ),
                    "bk": np.ascontiguousarray(
                        np.asarray(bk[g * 128 : (g + 1) * 128], np.float32).reshape(
                            D, 1
                        )
                    ),
                    "cosT": cosT,
                    "sinT": sinT,
                    "tri": tri,
                }
            )
    return in_maps


def combine_outputs(res, inputs):
    bv, Wo = np.asarray(inputs["bv"]), np.asarray(inputs["Wo"])
    out = np.zeros((2, T, C), dtype=np.float32)
    for c in range(8):
        g = c % 4
        out[c // 4] += np.asarray(res.results[c]["out"], dtype=np.float32)
        # v-bias contribution: softmax rows sum to 1, so bv adds the constant
        # row (bv tiled over the 4 q heads) @ Wo_group to every output row
        bv_tiled = np.tile(bv[g * 128 : (g + 1) * 128], NH).astype(np.float64)
        cvec = bv_tiled @ Wo[g * 512 : (g + 1) * 512, :].astype(np.float64)
        out[c // 4] += cvec.astype(np.float32)[None, :]
    return out


def kernel(x, Wq, bq, Wk, bk, Wv, bv, Wo):
    nc = _get_program()
    in_maps = make_in_maps(x, Wq, bq, Wk, bk, Wv, bv, Wo)
    res = run_bass_kernel_spmd(nc, in_maps, list(range(8)))
    return combine_outputs(res, {"bv": bv, "Wo": Wo})
